# revision 1
# baseline (speedup 1.0000x reference)
"""Contrastive (InfoNCE-style) loss kernel for 8 Trainium2 NeuronCores.

Reference computation:
    logits = (outputs @ targets.T) / (||o||_row * ||t||_col)   # [B, B]
    loss   = mean_i( logsumexp_j(logits[i, :]) - logits[i, i] )

Sharding: rows (outputs) split across 8 cores, 2048 rows each. Every core
receives the full row-normalized, transposed targets matrix with its
columns rotated by -core*2048, so each core's diagonal block lands at
columns [m*128, m*128+128) of the first 2048-column superchunk — the
program is identical across cores (pure SPMD, no core-id branching).

Per core, on device:
  - logit block [2048, 16384] = ohatT.T @ thatT via float32r matmuls
    (K=256 contracted as 2x128 partition chunks, fp32 PSUM accumulate)
  - cosine logits are in [-1, 1] so exp cannot overflow: plain sum-exp,
    no max-subtraction pass needed
  - exp on the scalar engine; row-sums per scheme (see _build_program)
  - diagonal extracted with an identity-mask multiply+reduce on the
    vector engine before exp overwrites the block

Host: row-normalize/transpose/rotate inputs (O(B*D) prep), final
log(sum_exp) - diag and the mean over 16384 rows (O(B) epilogue).
"""

import numpy as np

B = 16384
D = 256
NCORES = 8
S = B // NCORES          # 2048 rows per core
P = 128                  # partitions
M_TILES = S // P         # 16 row tiles per core
SC_W = 2048              # column superchunk width (4 PSUM banks)
N_SC = B // SC_W         # 8 superchunks
MM_N = 512               # matmul moving free dim (1 PSUM bank fp32)
NJ = SC_W // MM_N        # 4 matmuls per K-chunk per superchunk

# Row-sum scheme, chosen by TimelineSim modeling (see bench.py):
#   "accum":    exp in-place in PSUM, ACT accumulator does the row-sum
#   "dve_bf16": exp writes bf16 to SBUF, DVE reduce_sum does the row-sum
#   "split6":   5/6 chunks DVE-reduced from PSUM fp32, 1/6 ACT-accumulated
#   "fold2":    exp writes bf16 to SBUF; DVE tree-folds at bf16 2x rate
#               then reduces, keeping ACT exp-only
SCHEME = "fold2"
# schemes whose s_row is computed inline during the last superchunk pass
SCHEME_NEEDS_TAIL_REDUCE = {"fold2": False}

_PROGRAM_CACHE = {}
LAST_RESULTS = None      # BassKernelResults of the most recent run (for test.py)


def _build_program(scheme=None):
    import concourse.bacc as bacc
    import concourse.tile as tile
    from concourse import mybir

    scheme = scheme or SCHEME
    f32 = mybir.dt.float32
    f32r = mybir.dt.float32r
    bf16 = mybir.dt.bfloat16
    AF = mybir.ActivationFunctionType

    nc = bacc.Bacc(
        "TRN2",
        target_bir_lowering=False,
        debug=False,
        num_devices=NCORES,
    )

    ot = nc.dram_tensor("ot", [D, S], f32r, kind="ExternalInput").ap()
    tt = nc.dram_tensor("tt", [D, B], f32r, kind="ExternalInput").ap()
    ident = nc.dram_tensor("ident", [P, P], f32, kind="ExternalInput").ap()
    # columns 0..15: per-row sum of exp(logits); 16..31: diagonal logits
    vout = nc.dram_tensor("vout", [P, 2 * M_TILES], f32, kind="ExternalOutput").ap()

    with tile.TileContext(nc) as tc:
        with (
            tc.tile_pool(name="const", bufs=1) as const_pool,
            tc.tile_pool(name="tbuf", bufs=3) as tpool,
            tc.tile_pool(name="ebuf", bufs=3) as epool,
            tc.tile_pool(name="psum", bufs=2, space="PSUM") as psum_pool,
            tc.tile_pool(name="stats", bufs=1) as stats_pool,
        ):
            warm = const_pool.tile([P, 1], f32)
            nc.vector.memset(warm[:], 0.0)
            # pull the exp ACT-table load off the critical path
            nc.scalar.activation(out=warm[:], in_=warm[:], func=AF.Exp)

            # outputs^T, both K-chunks, resident for the whole kernel.
            # The modeled DMA engines are a shared resource that serves
            # transfers roughly in issue order, so issue order is laid out
            # to unblock the first (m=0, nb=0) chunk as early as possible:
            # m=0 output slices first, then the first targets superchunk
            # (issued below), then the remaining output columns.
            ot0 = const_pool.tile([P, S], f32r)
            ot1 = const_pool.tile([P, S], f32r)
            nc.gpsimd.dma_start(out=ot0[:, 0:P], in_=ot[0:P, 0:P])
            nc.gpsimd.dma_start(out=ot1[:, 0:P], in_=ot[P : 2 * P, 0:P])
            idt = const_pool.tile([P, P], f32)

            # per-(m, superchunk) exp row-sums and per-m diagonals
            s_parts = stats_pool.tile([P, M_TILES * N_SC], f32)
            s_extra = stats_pool.tile([P, NJ], f32)
            sd = stats_pool.tile([P, 2 * M_TILES], f32)
            s_row = sd[:, 0:M_TILES]
            d_sb = sd[:, M_TILES : 2 * M_TILES]
            junk = stats_pool.tile([P, P], f32)

            for nb in range(N_SC):
                t0 = tpool.tile([P, SC_W], f32r, tag="t0")
                t1 = tpool.tile([P, SC_W], f32r, tag="t1")
                c0 = nb * SC_W
                if nb == 0:
                    # chunked first load, split across both HWDGE queues
                    # (SP + Activation) so the first matmuls start ASAP.
                    # Small ot column pieces are interleaved so tile m's
                    # weights land just before chunk (m, 0) needs them.
                    for j in range(NJ):
                        sl = slice(j * MM_N, (j + 1) * MM_N)
                        dsl = slice(c0 + j * MM_N, c0 + (j + 1) * MM_N)
                        nc.sync.dma_start(out=t0[:, sl], in_=tt[0:P, dsl])
                        nc.scalar.dma_start(out=t1[:, sl], in_=tt[P : 2 * P, dsl])
                    nc.gpsimd.dma_start(out=idt[:], in_=ident[:])
                    for a, bnd in ((P, 512), (512, 1024), (1024, S)):
                        nc.sync.dma_start(out=ot0[:, a:bnd], in_=ot[0:P, a:bnd])
                        nc.scalar.dma_start(
                            out=ot1[:, a:bnd], in_=ot[P : 2 * P, a:bnd]
                        )
                else:
                    nc.sync.dma_start(out=t0[:], in_=tt[0:P, c0 : c0 + SC_W])
                    nc.sync.dma_start(out=t1[:], in_=tt[P : 2 * P, c0 : c0 + SC_W])
                for m in range(M_TILES):
                    ps = psum_pool.tile([P, SC_W], f32, tag="ps")
                    for j in range(NJ):
                        sl = slice(j * MM_N, (j + 1) * MM_N)
                        nc.tensor.matmul(
                            ps[:, sl],
                            ot0[:, m * P : (m + 1) * P],
                            t0[:, sl],
                            start=True,
                            stop=False,
                        )
                        nc.tensor.matmul(
                            ps[:, sl],
                            ot1[:, m * P : (m + 1) * P],
                            t1[:, sl],
                            start=False,
                            stop=True,
                        )
                    if nb == 0:
                        # rotated targets put this tile's diagonal at
                        # columns [m*P, m*P+P) of superchunk 0
                        nc.vector.tensor_mul(
                            junk[:], ps[:, m * P : (m + 1) * P], idt[:]
                        )
                        nc.vector.reduce_sum(
                            out=d_sb[:, m : m + 1],
                            in_=junk[:],
                            axis=mybir.AxisListType.X,
                        )
                    ci = m * N_SC + nb
                    col = slice(ci, ci + 1)
                    if scheme == "accum":
                        nc.scalar.activation(
                            out=ps[:], in_=ps[:], func=AF.Exp,
                            accum_out=s_parts[:, col],
                        )
                    elif scheme == "dve_bf16":
                        eo = epool.tile([P, SC_W], bf16, tag="eo")
                        nc.scalar.activation(out=eo[:], in_=ps[:], func=AF.Exp)
                        nc.vector.reduce_sum(
                            out=s_parts[:, col], in_=eo[:],
                            axis=mybir.AxisListType.X,
                        )
                    elif scheme == "split6":
                        if ci % 6 == 0:
                            nc.scalar.activation(
                                out=ps[:], in_=ps[:], func=AF.Exp,
                                accum_out=s_parts[:, col],
                            )
                        else:
                            nc.scalar.activation(out=ps[:], in_=ps[:], func=AF.Exp)
                            nc.vector.reduce_sum(
                                out=s_parts[:, col], in_=ps[:],
                                axis=mybir.AxisListType.X,
                            )
                    elif scheme == "fold2":
                        if nb == N_SC - 1 and m == M_TILES - 1:
                            # last chunk: ACT accumulator path — trims the
                            # DVE fold chain off the kernel tail
                            nc.scalar.activation(
                                out=ps[:], in_=ps[:], func=AF.Exp,
                                accum_out=s_parts[:, col],
                            )
                        else:
                            # exp -> bf16 SBUF; DVE folds halves twice at
                            # the bf16 2x rate, then one 512-wide reduce
                            eo = epool.tile([P, SC_W], bf16, tag="eo")
                            f1 = epool.tile([P, SC_W // 2], bf16, tag="f1")
                            f2 = epool.tile([P, SC_W // 4], bf16, tag="f2")
                            h = SC_W // 2
                            q = SC_W // 4
                            nc.scalar.activation(out=eo[:], in_=ps[:], func=AF.Exp)
                            nc.vector.tensor_add(f1[:], eo[:, 0:h], eo[:, h:SC_W])
                            nc.vector.tensor_add(f2[:], f1[:, 0:q], f1[:, q:h])
                            nc.vector.reduce_sum(
                                out=s_parts[:, col], in_=f2[:],
                                axis=mybir.AxisListType.X,
                            )
                    else:
                        raise ValueError(scheme)
                    if nb == N_SC - 1:
                        # row-tile m is complete: fold its 8 superchunk
                        # partial sums now instead of in a tail burst
                        nc.vector.reduce_sum(
                            out=s_row[:, m : m + 1],
                            in_=s_parts[:, m * N_SC : (m + 1) * N_SC],
                            axis=mybir.AxisListType.X,
                        )

            if SCHEME_NEEDS_TAIL_REDUCE.get(scheme, True):
                for m in range(M_TILES):
                    nc.vector.reduce_sum(
                        out=s_row[:, m : m + 1],
                        in_=s_parts[:, m * N_SC : (m + 1) * N_SC],
                        axis=mybir.AxisListType.X,
                    )
            nc.sync.dma_start(out=vout[:, 0 : 2 * M_TILES], in_=sd[:])

    nc.compile()
    return nc


def kernel(outputs: np.ndarray, targets: np.ndarray) -> np.ndarray:
    import os

    from concourse.bass_utils import run_bass_kernel_spmd

    global LAST_RESULTS

    o = np.ascontiguousarray(np.asarray(outputs, dtype=np.float32))
    t = np.ascontiguousarray(np.asarray(targets, dtype=np.float32))
    assert o.shape == (B, D) and t.shape == (B, D)

    o_hat = (o / np.linalg.norm(o, axis=1)[:, None]).astype(np.float32)
    t_hat = (t / np.linalg.norm(t, axis=1)[:, None]).astype(np.float32)
    eye = np.eye(P, dtype=np.float32)

    in_maps = []
    for c in range(NCORES):
        ot_c = np.ascontiguousarray(o_hat[c * S : (c + 1) * S].T)
        tt_c = np.ascontiguousarray(np.roll(t_hat, -c * S, axis=0).T)
        in_maps.append({"ot": ot_c, "tt": tt_c, "ident": eye})

    if "prog" not in _PROGRAM_CACHE:
        _PROGRAM_CACHE["prog"] = _build_program()
    nc = _PROGRAM_CACHE["prog"]

    trace = bool(os.environ.get("CONTRASTIVE_KERNEL_TRACE"))
    res = run_bass_kernel_spmd(
        nc, in_maps, core_ids=list(range(NCORES)), trace=trace
    )
    LAST_RESULTS = res

    rows = np.empty(B, dtype=np.float64)
    for c in range(NCORES):
        v = res.results[c]["vout"]  # [P, 2*M_TILES]
        s = v[:, 0:M_TILES].T.reshape(-1).astype(np.float64)
        d = v[:, M_TILES : 2 * M_TILES].T.reshape(-1).astype(np.float64)
        rows[c * S : (c + 1) * S] = np.log(s) - d

    loss = rows.mean()
    return np.asarray(loss, dtype=np.float32)

